# revision 43
# baseline (speedup 1.0000x reference)
"""Trainium2 Bass kernel for nn_Decoder (LSTM decoder with dot-product attention).

Strategy (8 NeuronCores, zero cross-core communication):
  Phase 1 -- the sequential LSTM recurrence is replicated on every core
    (batch-rolled per core so each core's local batches sit at rows 0..7).
    Per step: gates = h @ W_hh.T + b (PE, weight-stream bound), LSTM cell
    update (ACT/DVE), h_t appended to a DRAM history buffer.  Hardware For_i
    loop keeps the program small.
  Phase 2 -- attention + vocab projection are batch-sharded: the host gives
    each core its own 8 batches of encodings (natural and pre-transposed
    layouts).  Per batch: transpose h history, q~ = h @ (Wq.T Wk), scores
    against enc.T, exp (softmax deferred: normalizer applied as a per-row
    scale at the logits copy-out), av = exp @ enc, logits = av @ Wc.T + bc.

The host wrapper shards inputs, pre-transposes weights, and reassembles the
full [64, 512, 4096] output.
"""
import sys
import numpy as np

for _p in ('/opt/trn_rl_repo', '/root/.axon_site/_ro/trn_rl_repo'):
    if _p not in sys.path:
        sys.path.insert(0, _p)

import ml_dtypes
import concourse.bass as bass
import concourse.tile as tile
from concourse import bacc, masks, mybir
from concourse.bass_utils import run_bass_kernel_spmd

FP32 = mybir.dt.float32
FP32R = mybir.dt.float32r
BF16 = mybir.dt.bfloat16
AF = mybir.ActivationFunctionType


def _tf32(a):
    """Host-side round fp32 -> tf32 (float32r): keep 10 mantissa bits."""
    b = np.ascontiguousarray(a, np.float32).view(np.uint32)
    r = (b + np.uint32(0x1000)) & np.uint32(0xFFFFE000)
    return r.view(np.float32)

B, H, V = 64, 1024, 4096
NCORES = 8
BL = B // NCORES          # local batches per core
KT = H // 128             # 8 k-tiles
GC = 4 * H // 512         # 8 gate chunks of 512
VT = V // 512             # 8 vocab chunks of 512


def build_nc(S, logits_bf16=True, s_run=None, bl_run=None, hist_write=True, p2_rep=1, p1_only=False):
    """Build the Bass program for sequence length S (S % 128 == 0)."""
    TT = S // 128          # number of 128-row t-tiles in phase 2
    s_run = S if s_run is None else s_run
    bl_run = BL if bl_run is None else bl_run
    SC = min(S, 512)       # free-dim chunk for S-wide matmuls
    NSC = S // SC
    nc = bacc.Bacc("TRN2", target_bir_lowering=False, debug=False,
                   enable_asserts=True, num_devices=NCORES)

    ldt = BF16 if logits_bf16 else FP32

    # ---- I/O ----
    h0T = nc.dram_tensor("h0T", [H, B], FP32R, kind="ExternalInput")
    h0n = nc.dram_tensor("h0n", [B, H], FP32, kind="ExternalInput")
    c0 = nc.dram_tensor("c0", [B, H], FP32, kind="ExternalInput")
    whhT = nc.dram_tensor("whhT", [H, 4 * H], FP32R, kind="ExternalInput")
    bgb = nc.dram_tensor("bgb", [B, 4 * H], FP32R, kind="ExternalInput")
    sdt = ldt if logits_bf16 else FP32R   # dtype for scores-path operands
    if not p1_only:
        wqk = nc.dram_tensor("wqk", [H, H], FP32R, kind="ExternalInput")
        wcT = nc.dram_tensor("wcT", [H, V], ldt, kind="ExternalInput")
        bcb = nc.dram_tensor("bcb", [1, V], ldt, kind="ExternalInput")
        encl = nc.dram_tensor("encl", [BL * S, H], ldt, kind="ExternalInput")
        enclT = nc.dram_tensor("enclT", [BL * H, S], sdt, kind="ExternalInput")
        outl = nc.dram_tensor("outl", [BL * S, V], ldt, kind="ExternalOutput")
    else:
        outl = nc.dram_tensor("outl", [B, H], FP32, kind="ExternalOutput")

    hist = nc.dram_tensor("hist", [S * BL, H], FP32)   # local h_t history

    with tile.TileContext(nc) as tc:
        with tc.tile_pool(name="glob", bufs=1) as glob:
            id_sb = glob.tile([128, 128], FP32, tag="id")
            masks.make_identity(nc, id_sb[:])
            id_bf = glob.tile([128, 128], BF16, tag="idbf")
            masks.make_identity(nc, id_bf[:])

            # ---------------- Phase 1: recurrence ----------------
            # Per-step schedule keeps the PE dense (HAM stays at K=8/8):
            #   A(0,2,4) kt0-3 | T1' (prev step h-half1 transposes) | A kt4-7
            #   | B(1,3,5) | C(6) | D(7) | T0 (h-half0) | next step ...
            # Bias enters via a K=1 ones-matmul seeding each PSUM chunk, so
            # ACT reads gates straight from PSUM (no DVE bias adds).
            with (
                tc.tile_pool(name="p1", bufs=1) as p1,
                tc.tile_pool(name="p1ps", bufs=6, space="PSUM") as p1ps,
                tc.tile_pool(name="p1pst", bufs=2, space="PSUM") as p1pst,
            ):
                whh_sb = p1.tile([128, KT * 4 * H], FP32R, tag="whh")
                for kt in range(KT):
                    nc.sync.dma_start(whh_sb[:, kt * 4 * H:(kt + 1) * 4 * H],
                                      whhT[kt * 128:(kt + 1) * 128, :])
                bgb_sb = p1.tile([B, 4 * H], FP32R, tag="bgb")
                nc.sync.dma_start(bgb_sb[:], bgb[:])
                ones_f32 = p1.tile([1, B], FP32, tag="ones_f")
                nc.vector.memset(ones_f32[:], 1.0)
                ones_sb = p1.tile([1, B], FP32R, tag="ones")
                nc.vector.tensor_copy(ones_sb[:], ones_f32[:])
                hT = p1.tile([128, KT * B], FP32R, tag="hT")
                for kt in range(KT):
                    nc.sync.dma_start(hT[:, kt * B:(kt + 1) * B],
                                      h0T[kt * 128:(kt + 1) * 128, :])
                c_sb = p1.tile([B, H], FP32, tag="c")
                nc.sync.dma_start(c_sb[:], c0[:])
                gsb = p1.tile([B, 4 * H], FP32, tag="gsb")
                tmp = p1.tile([B, H], FP32, tag="tmp")
                th = p1.tile([B, H], FP32, tag="th")
                hnat = p1.tile([B, H], FP32, tag="hnat")
                # seed hnat half1 so step 0's deferred T1 rewrites h0 values
                nc.sync.dma_start(hnat[:], h0n[:])

                hist_r = hist[:].rearrange("(t b) k -> t b k", b=BL)

                # chunk layout (512-wide): 0,1=i 2,3=f 4,5=g~ 6,7=o
                CH_ACT = {0: AF.Sigmoid, 1: AF.Sigmoid, 2: AF.Sigmoid,
                          3: AF.Sigmoid, 4: AF.Tanh, 5: AF.Tanh,
                          6: AF.Sigmoid, 7: AF.Sigmoid}

                def _alloc(chunks):
                    return {g: p1ps.tile([B, 512], FP32, tag="gates",
                                         name=f"gch{g}")
                            for g in chunks}

                def _bias_mm(pss, chunks):
                    for g in chunks:
                        nc.tensor.matmul(
                            pss[g][:], ones_sb[:],
                            bgb_sb[0:1, g * 512:(g + 1) * 512],
                            start=True, stop=False)

                def _mm(pss, chunks, kts, stop, start=False):
                    kts = list(kts)
                    for kt in kts:
                        for g in chunks:
                            nc.tensor.matmul(
                                pss[g][:],
                                hT[:, kt * B:(kt + 1) * B],
                                whh_sb[:, kt * 4 * H + g * 512: kt * 4 * H + (g + 1) * 512],
                                start=(start and kt == kts[0]),
                                stop=(stop and kt == kts[-1]))

                def _acts(pss, chunks):
                    # o-chunks (6,7) were bias-seeded in PSUM: ACT reads PSUM.
                    # others: DVE adds bias (overlaps PE), then ACT in place.
                    for g in chunks:
                        sl = slice(g * 512, (g + 1) * 512)
                        if g >= 6:
                            nc.scalar.activation(gsb[:, sl], pss[g][:], CH_ACT[g])
                        else:
                            nc.vector.tensor_add(gsb[:, sl], pss[g][:], bgb_sb[:, sl])
                            nc.scalar.activation(gsb[:, sl], gsb[:, sl], CH_ACT[g])

                def _cell_half(hh):
                    o = hh * 512
                    nc.vector.tensor_mul(c_sb[:, o:o + 512],
                                         gsb[:, H + o:H + o + 512], c_sb[:, o:o + 512])
                    nc.vector.tensor_mul(tmp[:, o:o + 512],
                                         gsb[:, o:o + 512], gsb[:, 2 * H + o:2 * H + o + 512])
                    nc.vector.tensor_add(c_sb[:, o:o + 512],
                                         c_sb[:, o:o + 512], tmp[:, o:o + 512])
                    nc.scalar.activation(th[:, o:o + 512], c_sb[:, o:o + 512], AF.Tanh)

                def _trans_half(hh):
                    # 4 transposes of hnat half -> one PSUM tile, 1 wide copy
                    pst = p1pst.tile([128, 4 * B], FP32, tag="tp",
                                     name=f"tp{hh}")
                    for j in range(4):
                        kt = 4 * hh + j
                        nc.tensor.transpose(pst[:, j * B:(j + 1) * B],
                                            hnat[:, kt * 128:(kt + 1) * 128],
                                            id_sb[0:B, 0:B])
                    nc.vector.tensor_copy(hT[:, 4 * hh * B:(4 * hh + 4) * B],
                                          pst[:])

                def _step(t):
                    pssA = _alloc((0, 2, 4))
                    _mm(pssA, (0, 2, 4), range(0, 4), stop=False, start=True)
                    _trans_half(1)          # deferred: prev step's h half1
                    _mm(pssA, (0, 2, 4), range(4, 8), stop=True)
                    _acts(pssA, (0, 2, 4))
                    _cell_half(0)
                    pssB = _alloc((1, 3, 5))
                    _mm(pssB, (1, 3, 5), range(8), stop=True, start=True)
                    _acts(pssB, (1, 3, 5))
                    _cell_half(1)
                    pssC = _alloc((6,))
                    _bias_mm(pssC, (6,))
                    _mm(pssC, (6,), range(8), stop=True)
                    _acts(pssC, (6,))
                    nc.vector.tensor_mul(hnat[:, 0:512],
                                         gsb[:, 3 * H:3 * H + 512], th[:, 0:512])
                    pssD = _alloc((7,))
                    _bias_mm(pssD, (7,))
                    _mm(pssD, (7,), range(8), stop=True)
                    _trans_half(0)
                    _acts(pssD, (7,))
                    nc.vector.tensor_mul(hnat[:, 512:1024],
                                         gsb[:, 3 * H + 512:4 * H], th[:, 512:1024])
                    if hist_write:
                        nc.sync.dma_start(hist_r[bass.ds(t, 1), :, :].opt(),
                                          hnat[0:BL, :])

                UN = 32
                assert s_run % UN == 0
                with tc.For_i(0, s_run // UN, 1) as it:
                    for u in range(UN):
                        _step(it * UN + u)
                if p1_only:
                    nc.sync.dma_start(outl[:], c_sb[:])

            if p1_only:
                rep_cm = None
            # ---------------- Phase 2: attention + logits ----------------
            if p1_only:
                pass
            else:
             with (
                tc.tile_pool(name="p2", bufs=1) as p2,
                tc.tile_pool(name="p2b", bufs=2) as p2b,
                tc.tile_pool(name="p2d", bufs=3) as p2d,
                tc.tile_pool(name="enc4", bufs=TT + 2) as enc4,
                tc.tile_pool(name="p2ps", bufs=4, space="PSUM") as p2ps,
                tc.tile_pool(name="p2pst", bufs=2, space="PSUM") as p2pst,
            ):
                wqk_sb = p2.tile([128, KT * H], FP32R, tag="wqk")
                for kt in range(KT):
                    nc.sync.dma_start(wqk_sb[:, kt * H:(kt + 1) * H],
                                      wqk[kt * 128:(kt + 1) * 128, :])
                wc_sb = p2.tile([128, KT * V], ldt, tag="wc")
                for kt in range(KT):
                    nc.sync.dma_start(wc_sb[:, kt * V:(kt + 1) * V],
                                      wcT[kt * 128:(kt + 1) * 128, :])
                bcb_sb = p2.tile([1, V], ldt, tag="bcb")
                nc.sync.dma_start(bcb_sb[:], bcb[:])

                hist_r2 = hist[:].rearrange("(t b) k -> t b k", b=BL)
                encl_r = encl[:].rearrange("(b s) k -> b s k", s=S)
                enclT_r = enclT[:].rearrange("(b k) s -> b k s", k=H)
                outl_r = outl[:].rearrange("(b s) v -> b s v", s=S)

                rep_cm = tc.For_i(0, p2_rep, 1) if p2_rep > 1 else None
                if rep_cm is not None:
                    rep_cm.__enter__()
                for b in range(bl_run):
                    # per-batch tiles from bufs=2 pool -> batches pipeline
                    hT_b = p2b.tile([128, KT * S], FP32R, tag="hTb")
                    qT_b = p2b.tile([128, KT * S], sdt, tag="qTb")
                    exp_b = p2b.tile([128, TT * 512], ldt, tag="expb")
                    expT_b = p2b.tile([128, TT * 512], ldt, tag="expTb")
                    avT_b = p2b.tile([128, KT * S], ldt, tag="avTb")
                    sums = p2b.tile([128, TT], FP32, tag="sums")
                    invs = p2b.tile([128, TT], FP32, tag="invs")
                    sumsT = p2b.tile([1, TT * 128], ldt, tag="sumsT")
                    # A: hT_b[k, t] = transpose of h history for batch b
                    for kt in range(KT):
                        for tt in range(TT):
                            hin = p2d.tile([128, 128], FP32, tag="hin")
                            nc.sync.dma_start(
                                hin[:],
                                hist_r2[bass.ds(tt * 128, 128), bass.ds(b, 1),
                                        kt * 128:(kt + 1) * 128].opt())
                            pst = p2pst.tile([128, 128], FP32, tag="tpA")
                            nc.tensor.transpose(pst[:], hin[:], id_sb[:])
                            nc.vector.tensor_copy(
                                hT_b[:, kt * S + tt * 128: kt * S + tt * 128 + 128],
                                pst[:])
                    # B: qT_b[j, t] = sum_k wqk[k, j] hT_b[k, t]
                    for jt in range(KT):
                        for sc in range(NSC):
                            psb = p2ps.tile([128, SC], FP32, tag="ps")
                            for kt in range(KT):
                                nc.tensor.matmul(
                                    psb[:],
                                    wqk_sb[:, kt * H + jt * 128: kt * H + (jt + 1) * 128],
                                    hT_b[:, kt * S + sc * SC: kt * S + (sc + 1) * SC],
                                    start=(kt == 0), stop=(kt == KT - 1))
                            nc.vector.tensor_copy(
                                qT_b[:, jt * S + sc * SC: jt * S + (sc + 1) * SC],
                                psb[:])
                    # C: scores -> exp -> row sums  (scores[t, s]; TT psum banks live)
                    pscs = [p2ps.tile([128, SC], FP32, tag="ps", name=f"psc{_tt}")
                            for _tt in range(TT)]
                    for jt in range(KT):
                        encT_t = p2d.tile([128, SC], sdt, tag="encTt")
                        nc.sync.dma_start(
                            encT_t[:, 0:S],
                            enclT_r[bass.ds(b, 1), jt * 128:(jt + 1) * 128, :].opt())
                        for tt in range(TT):
                            nc.tensor.matmul(
                                pscs[tt][:, 0:S],
                                qT_b[:, jt * S + tt * 128: jt * S + tt * 128 + 128],
                                encT_t[:, 0:S],
                                start=(jt == 0), stop=(jt == KT - 1))
                    for tt in range(TT):
                        nc.scalar.activation(exp_b[:, tt * 512: tt * 512 + S],
                                             pscs[tt][:, 0:S], AF.Exp)
                        nc.vector.reduce_sum(sums[:, tt:tt + 1],
                                             exp_b[:, tt * 512: tt * 512 + S],
                                             axis=mybir.AxisListType.X)
                    nc.vector.reciprocal(invs[:], sums[:])
                    # sumsT[0, tt*128+t]: for seeding F's PSUM with sums[t]*bc[v]
                    for tt in range(TT):
                        pstS = p2pst.tile([1, 128], FP32, tag="tpA")
                        nc.tensor.transpose(pstS[:], sums[:, tt:tt + 1], id_sb[:])
                        nc.vector.tensor_copy(
                            sumsT[:, tt * 128:(tt + 1) * 128], pstS[:])
                    # expT: [s, t] tiles
                    for tt in range(TT):
                        for st in range(TT):
                            pst2 = p2pst.tile([128, 128], ldt, tag="tpE")
                            nc.tensor.transpose(
                                pst2[:],
                                exp_b[:, tt * 512 + st * 128: tt * 512 + st * 128 + 128],
                                id_bf[:] if logits_bf16 else id_sb[:])
                            nc.vector.tensor_copy(
                                expT_b[:, st * 512 + tt * 128: st * 512 + tt * 128 + 128],
                                pst2[:])
                    # E: avT_b[k, t] = sum_s enc[b, s, k] expT[s, t]
                    enc_tiles = []
                    for st in range(TT):
                        et = enc4.tile([128, H], ldt, tag="enc")
                        nc.sync.dma_start(
                            et[:],
                            encl_r[bass.ds(b, 1), bass.ds(st * 128, 128), :].opt())
                        enc_tiles.append(et)
                    for kt in range(KT):
                        for sc in range(NSC):
                            pse = p2ps.tile([128, SC], FP32, tag="ps")
                            for st in range(TT):
                                nc.tensor.matmul(
                                    pse[:],
                                    enc_tiles[st][:, kt * 128:(kt + 1) * 128],
                                    expT_b[:, st * 512 + sc * SC: st * 512 + (sc + 1) * SC],
                                    start=(st == 0), stop=(st == TT - 1))
                            nc.vector.tensor_copy(
                                avT_b[:, kt * S + sc * SC: kt * S + (sc + 1) * SC],
                                pse[:])
                    # F: logits[t, v] = (sums[t]*bc[v] + sum_k avT[k,t] wcT[k,v]) * inv[t]
                    for vt in range(VT):
                        for tt in range(TT):
                            psf = p2ps.tile([128, 512], FP32, tag="ps")
                            nc.tensor.matmul(
                                psf[:], sumsT[:, tt * 128:(tt + 1) * 128],
                                bcb_sb[:, vt * 512:(vt + 1) * 512],
                                start=True, stop=False)
                            for kt in range(KT):
                                nc.tensor.matmul(
                                    psf[:],
                                    avT_b[:, kt * S + tt * 128: kt * S + tt * 128 + 128],
                                    wc_sb[:, kt * V + vt * 512: kt * V + (vt + 1) * 512],
                                    start=False, stop=(kt == KT - 1))
                            lout = p2d.tile([128, 512], ldt, tag="lout")
                            nc.scalar.activation(lout[:], psf[:], AF.Copy,
                                                 scale=invs[:, tt:tt + 1])
                            nc.sync.dma_start(
                                outl_r[bass.ds(b, 1), tt * 128:(tt + 1) * 128,
                                       vt * 512:(vt + 1) * 512].opt(),
                                lout[:])

                if rep_cm is not None:
                    rep_cm.__exit__(None, None, None)

    nc.compile()
    return nc


_NC_CACHE = {}


def _get_nc(S, logits_bf16=True, s_run=None, bl_run=None, hist_write=True, p2_rep=1):
    key = (S, logits_bf16, s_run, bl_run, hist_write, p2_rep)
    if key not in _NC_CACHE:
        _NC_CACHE[key] = build_nc(S, logits_bf16, s_run, bl_run, hist_write, p2_rep)
    return _NC_CACHE[key]


def _prep_inputs(features, encodings, h0, c0, W_hh, b_ih, b_hh, Wq, Wk, Wc, bc,
                 S, logits_bf16=True):
    f32 = np.float32
    ldt = ml_dtypes.bfloat16 if logits_bf16 else f32
    enc = np.asarray(encodings, f32)[:, :S, :]
    whhT = _tf32(np.asarray(W_hh, f32).T)
    bg = (np.asarray(b_ih, f32) + np.asarray(b_hh, f32))
    bgb = _tf32(np.broadcast_to(bg, (B, 4 * H)))
    wqk = _tf32(np.asarray(Wq, f32).T @ np.asarray(Wk, f32))
    wcT = np.ascontiguousarray(np.asarray(Wc, f32).T).astype(ldt)
    bcb = np.asarray(bc, f32).reshape(1, V).astype(ldt)
    encT = np.transpose(enc, (0, 2, 1))
    encT = encT.astype(ldt) if logits_bf16 else _tf32(encT)
    h0 = np.asarray(h0, f32)
    c0 = np.asarray(c0, f32)

    in_maps = []
    for core in range(NCORES):
        bb = core * BL
        h0r = np.roll(h0, -bb, axis=0)
        c0r = np.roll(c0, -bb, axis=0)
        in_maps.append({
            "h0T": np.ascontiguousarray(h0r.T),
            "h0n": np.ascontiguousarray(h0r),
            "c0": np.ascontiguousarray(c0r),
            "whhT": whhT,
            "bgb": bgb,
            "wqk": wqk,
            "wcT": wcT,
            "bcb": bcb,
            "encl": np.ascontiguousarray(
                enc[bb:bb + BL].reshape(BL * S, H)).astype(ldt),
            "enclT": np.ascontiguousarray(encT[bb:bb + BL].reshape(BL * H, S)),
        })
    return in_maps


LAST_EXEC_NS = None
LAST_TRACE_DIR = None


def run_S(S, logits_bf16=True, trace=False, **inputs):
    """Run the kernel at sequence length S; returns [B, S, V] float32."""
    global LAST_EXEC_NS, LAST_TRACE_DIR
    nc = _get_nc(S, logits_bf16)
    in_maps = _prep_inputs(S=S, logits_bf16=logits_bf16, **inputs)
    kw = {}
    if trace:
        import tempfile
        LAST_TRACE_DIR = tempfile.mkdtemp(prefix="bass_trace_")
        kw = dict(trace=True, tmpdir=LAST_TRACE_DIR)
    res = run_bass_kernel_spmd(nc, in_maps, core_ids=list(range(NCORES)), **kw)
    if trace:
        LAST_EXEC_NS = res.exec_time_ns
    out = np.empty((B, S, V), np.float32)
    for core in range(NCORES):
        out[core * BL:(core + 1) * BL] = (
            res.results[core]["outl"].astype(np.float32).reshape(BL, S, V))
    return out


def kernel(**inputs):
    return run_S(512, **inputs)

